# revision 7
# baseline (speedup 1.0000x reference)
"""EMA (exponential moving average) linear recurrence on 8 trn2 NeuronCores.

y[0] = x[0]; y[t] = s*x[t] + (1-s)*y[t-1],  s = 0.3, x: (64, 4096, 256) fp32.

Algorithm: with a = 1-s = 0.7, a^128 ~ 1.6e-20, so history beyond 128 steps is
far below fp32 resolution. Chunk T into blocks of L=128 and write the scan as a
blocked FIR evaluated on the TensorEngine:

    y_c = M @ x_c + P @ x_{c-1}        (chunk 0: y_0 = M0 @ x_0)

with constant 128x128 matrices
    M[i,j]  = s * a^(i-j)   (j <= i),   M0 = M with column 0 scaled to a^i
    P[i,j]  = s * a^(i+128-j)           (dropped terms <= s*a^256 ~ 1e-40)

Sharding: batch B=64 split across the 8 cores (8 rows each); the recurrence is
along T only, so no cross-core communication is needed.

Precision/bandwidth: the harness gate is rel_err < 2e-2, so the kernel runs
fully in fp16: the host casts x to fp16 and transposes each core's shard to a
time-major [T, BC*D] layout (4 KiB contiguous DMA lines), the device computes
fp16 matmuls with fp32 PSUM accumulation, and writes y back as fp16; the host
upcasts/transposes to the fp32 output. End-to-end error vs the fp32 reference
is ~3e-4. This halves HBM traffic vs fp32 I/O (32 MiB/core total) and needs
just 2 PE passes/chunk, leaving the kernel DMA-bound near the ~332 GB/s/core
roofline.
"""
import numpy as np

import concourse.bacc as bacc
import concourse.mybir as mybir
from concourse import tile
from concourse.bass_utils import run_bass_kernel_spmd

S = 0.3
A = 1.0 - S
B, T, D = 64, 4096, 256
NCORES = 8
BC = B // NCORES          # 8 batch rows per core
L = 128                   # chunk length along T == matmul contraction dim
NCH = T // L              # 32 chunks
CB = BC * D               # 2048 free elements per chunk
NSL = CB // 512           # 4 matmul slices (one PSUM bank each)

f32 = mybir.dt.float32
f16 = mybir.dt.float16

_nc_cache = []


def _weights():
    i = np.arange(L, dtype=np.float64)[:, None]
    j = np.arange(L, dtype=np.float64)[None, :]
    M = np.where(j <= i, S * A ** (i - j), 0.0)
    M0 = M.copy()
    M0[:, 0] = A ** i[:, 0]
    P = S * A ** (i + L - j)
    # lhsT layout [K, M_out] = W.T, fp16
    return [np.ascontiguousarray(w.T.astype(np.float16)) for w in (M0, M, P)]


def _build():
    nc = bacc.Bacc("TRN2", target_bir_lowering=False, debug=False)
    x = nc.dram_tensor("x", [T, CB], f16, kind="ExternalInput").ap()
    # all three weight matrices in one tensor -> one DMA at kernel start
    wall = nc.dram_tensor("wall", [L, 3 * L], f16, kind="ExternalInput").ap()
    y = nc.dram_tensor("y", [T, CB], f16, kind="ExternalOutput").ap()

    with tile.TileContext(nc) as tc, \
         tc.tile_pool(name="w", bufs=1) as wpool, \
         tc.tile_pool(name="xs", bufs=12) as xpool, \
         tc.tile_pool(name="ys", bufs=6) as ypool, \
         tc.tile_pool(name="ps", bufs=2, space="PSUM") as pspool:
        wall_t = wpool.tile([L, 3 * L], f16)
        # first in the sync-ring queue: small, lands before chunk 0
        nc.sync.dma_start(wall_t[:], wall[:])
        wm0 = wall_t[:, 0 * L:1 * L]
        wm = wall_t[:, 1 * L:2 * L]
        wp = wall_t[:, 2 * L:3 * L]

        def load(c):
            xt = xpool.tile([L, CB], f16, name=f"xt{c}", tag="xt")
            src = x[c * L:(c + 1) * L, :]
            if c == 0:
                # chunk 0 gates PE start: pipeline it at 512-element slices
                for n in range(NSL):
                    sl = slice(n * 512, (n + 1) * 512)
                    nc.sync.dma_start(xt[:, sl], src[:, sl])
            else:
                nc.sync.dma_start(xt[:], src)
            return xt

        tiles = {0: load(0)}
        prev = None
        for c in range(NCH):
            # issue next chunk's load before this chunk's matmuls
            if c + 1 < NCH:
                tiles[c + 1] = load(c + 1)
            xt = tiles.pop(c)

            ps = pspool.tile([L, CB], f32)
            m = wm0 if c == 0 else wm
            # grouped by stationary weight to allow weight-load reuse
            for n in range(NSL):
                nc.tensor.matmul(
                    ps[:, n * 512:(n + 1) * 512], m,
                    xt[:, n * 512:(n + 1) * 512],
                    start=True, stop=(c == 0),
                )
            if c > 0:
                for n in range(NSL):
                    nc.tensor.matmul(
                        ps[:, n * 512:(n + 1) * 512], wp,
                        prev[:, n * 512:(n + 1) * 512],
                        start=False, stop=True,
                    )

            # evac PSUM->SBUF(fp16): ACT takes the low half, DVE the high half
            # (halves the evac latency gating PSUM reuse); the output DMA is
            # one full-chunk trigger on the otherwise-idle gpsimd queue, so
            # neither the input (sync) nor the evac (scalar) sequencer ever
            # waits on output descriptor generation.
            yt = ypool.tile([L, CB], f16)
            dst = y[c * L:(c + 1) * L, :]
            if c >= NCH - 2:
                # drain the tail at slice granularity to cut end latency
                for n in range(NSL):
                    sl = slice(n * 512, (n + 1) * 512)
                    if n % 2 == 0:
                        nc.scalar.copy(yt[:, sl], ps[:, sl])
                    else:
                        nc.vector.tensor_copy(yt[:, sl], ps[:, sl])
                    nc.gpsimd.dma_start(dst[:, sl], yt[:, sl])
            else:
                nc.scalar.copy(yt[:, :CB // 2], ps[:, :CB // 2])
                nc.vector.tensor_copy(yt[:, CB // 2:], ps[:, CB // 2:])
                nc.gpsimd.dma_start(dst, yt[:])
            prev = xt
    nc.compile()
    return nc


def get_nc():
    if not _nc_cache:
        _nc_cache.append(_build())
    return _nc_cache[0]


def make_in_maps(x: np.ndarray):
    x = np.asarray(x)
    assert x.shape == (B, T, D)
    wall = np.ascontiguousarray(np.concatenate(_weights(), axis=1))
    maps = []
    for i in range(NCORES):
        xc = x[i * BC:(i + 1) * BC]                   # [BC, T, D] fp32
        xc = np.ascontiguousarray(
            xc.transpose(1, 0, 2).reshape(T, CB).astype(np.float16)
        )
        maps.append({"x": xc, "wall": wall})
    return maps


def assemble(res) -> np.ndarray:
    out = np.empty((B, T, D), dtype=np.float32)
    for i in range(NCORES):
        yc = res[i]["y"].astype(np.float32).reshape(T, BC, D)
        out[i * BC:(i + 1) * BC] = yc.transpose(1, 0, 2)
    return out


def kernel(x: np.ndarray) -> np.ndarray:
    res = run_bass_kernel_spmd(
        get_nc(), make_in_maps(x), list(range(NCORES))
    ).results
    return assemble(res)


# revision 12
# speedup vs baseline: 1.0503x; 1.0503x over previous
"""EMA (exponential moving average) linear recurrence on 8 trn2 NeuronCores.

y[0] = x[0]; y[t] = s*x[t] + (1-s)*y[t-1],  s = 0.3, x: (64, 4096, 256) fp32.

Algorithm: with a = 1-s = 0.7, a^128 ~ 1.6e-20, so history beyond 128 steps is
far below fp32 resolution. Chunk T into blocks of L=128 and write the scan as a
blocked FIR evaluated on the TensorEngine:

    y_c = M @ x_c + P @ x_{c-1}        (chunk 0: y_0 = M0 @ x_0)

with constant 128x128 matrices
    M[i,j]  = s * a^(i-j)   (j <= i),   M0 = M with column 0 scaled to a^i
    P[i,j]  = s * a^(i+128-j)           (dropped terms <= s*a^256 ~ 1e-40)

Sharding: batch B=64 split across the 8 cores (8 rows each); the recurrence is
along T only, so no cross-core communication is needed.

Precision/bandwidth: the harness gate is rel_err < 2e-2, so the kernel runs
fully in fp16: the host casts x to fp16 and transposes each core's shard to a
time-major [T, BC*D] layout (4 KiB contiguous DMA lines), the device computes
fp16 matmuls with fp32 PSUM accumulation, and writes y back as fp16; the host
upcasts/transposes to the fp32 output. End-to-end error vs the fp32 reference
is ~3e-4. This halves HBM traffic vs fp32 I/O (32 MiB/core total) and needs
just 2 PE passes/chunk, leaving the kernel DMA-bound near the ~332 GB/s/core
roofline.
"""
import numpy as np

import concourse.bacc as bacc
import concourse.mybir as mybir
from concourse import tile
from concourse.bass_utils import run_bass_kernel_spmd

S = 0.3
A = 1.0 - S
B, T, D = 64, 4096, 256
NCORES = 8
BC = B // NCORES          # 8 batch rows per core
L = 128                   # chunk length along T == matmul contraction dim
NCH = T // L              # 32 chunks
CB = BC * D               # 2048 free elements per chunk
NSL = CB // 512           # 4 matmul slices (one PSUM bank each)

f32 = mybir.dt.float32
f16 = mybir.dt.float16

_nc_cache = []


def _weights():
    i = np.arange(L, dtype=np.float64)[:, None]
    j = np.arange(L, dtype=np.float64)[None, :]
    M = np.where(j <= i, S * A ** (i - j), 0.0)
    M0 = M.copy()
    M0[:, 0] = A ** i[:, 0]
    P = S * A ** (i + L - j)
    # lhsT layout [K, M_out] = W.T, fp16
    return [np.ascontiguousarray(w.T.astype(np.float16)) for w in (M0, M, P)]


def _build():
    nc = bacc.Bacc("TRN2", target_bir_lowering=False, debug=False)
    x = nc.dram_tensor("x", [T, CB], f16, kind="ExternalInput").ap()
    # all three weight matrices in one tensor -> one DMA at kernel start
    wall = nc.dram_tensor("wall", [L, 3 * L], f16, kind="ExternalInput").ap()
    y = nc.dram_tensor("y", [T, CB], f16, kind="ExternalOutput").ap()

    with tile.TileContext(nc) as tc, \
         tc.tile_pool(name="w", bufs=1) as wpool, \
         tc.tile_pool(name="xs", bufs=12) as xpool, \
         tc.tile_pool(name="ys", bufs=6) as ypool, \
         tc.tile_pool(name="ps", bufs=2, space="PSUM") as pspool:
        wall_t = wpool.tile([L, 3 * L], f16)
        # first in the sync-ring queue: small, lands before chunk 0
        nc.sync.dma_start(wall_t[:], wall[:])
        wm0 = wall_t[:, 0 * L:1 * L]
        wm = wall_t[:, 1 * L:2 * L]
        wp = wall_t[:, 2 * L:3 * L]

        def load(c):
            xt = xpool.tile([L, CB], f16, name=f"xt{c}", tag="xt")
            src = x[c * L:(c + 1) * L, :]
            if c == 0:
                # chunk 0 gates PE start: pipeline it at 512-element slices
                for n in range(NSL):
                    sl = slice(n * 512, (n + 1) * 512)
                    nc.sync.dma_start(xt[:, sl], src[:, sl])
            else:
                nc.sync.dma_start(xt[:], src)
            return xt

        tiles = {0: load(0)}
        prev = None
        for c in range(NCH):
            # issue next chunk's load before this chunk's matmuls
            if c + 1 < NCH:
                tiles[c + 1] = load(c + 1)
            xt = tiles.pop(c)

            # two half-chunk PSUM tiles (2 banks each, pool bufs=4): PSUM
            # frees at half-chunk granularity so the matmul->evac->free loop
            # keeps pace with the input stream instead of lagging into a
            # long output-only drain phase.
            ph = [pspool.tile([L, CB // 2], f32, name=f"ps{c}_{h}", tag=f"ps{h}")
                  for h in range(2)]
            m = wm0 if c == 0 else wm
            # grouped by stationary weight to allow weight-load reuse
            for n in range(NSL):
                nc.tensor.matmul(
                    ph[n // 2][:, (n % 2) * 512:(n % 2) * 512 + 512], m,
                    xt[:, n * 512:(n + 1) * 512],
                    start=True, stop=(c == 0),
                )
            if c > 0:
                for n in range(NSL):
                    nc.tensor.matmul(
                        ph[n // 2][:, (n % 2) * 512:(n % 2) * 512 + 512], wp,
                        prev[:, n * 512:(n + 1) * 512],
                        start=False, stop=True,
                    )

            # evac PSUM->SBUF(fp16): ACT takes the low half, DVE the high
            # half; one full-chunk output DMA (4 KiB descriptor runs) with
            # the trigger alternating between the scalar and gpsimd rings.
            yt = ypool.tile([L, CB], f16)
            dst = y[c * L:(c + 1) * L, :]
            if c >= NCH - 2:
                # drain the tail at slice granularity to cut end latency
                for n in range(NSL):
                    sl = slice(n * 512, (n + 1) * 512)
                    hsl = slice((n % 2) * 512, (n % 2) * 512 + 512)
                    if n % 2 == 0:
                        nc.scalar.copy(yt[:, sl], ph[n // 2][:, hsl])
                        nc.scalar.dma_start(dst[:, sl], yt[:, sl])
                    else:
                        nc.vector.tensor_copy(yt[:, sl], ph[n // 2][:, hsl])
                        nc.gpsimd.dma_start(dst[:, sl], yt[:, sl])
            else:
                nc.scalar.copy(yt[:, :CB // 2], ph[0][:])
                nc.vector.tensor_copy(yt[:, CB // 2:], ph[1][:])
                if c % 2 == 0:
                    nc.scalar.dma_start(dst, yt[:])
                else:
                    nc.gpsimd.dma_start(dst, yt[:])
            prev = xt
    nc.compile()
    return nc


def get_nc():
    if not _nc_cache:
        _nc_cache.append(_build())
    return _nc_cache[0]


def make_in_maps(x: np.ndarray):
    x = np.asarray(x)
    assert x.shape == (B, T, D)
    wall = np.ascontiguousarray(np.concatenate(_weights(), axis=1))
    maps = []
    for i in range(NCORES):
        xc = x[i * BC:(i + 1) * BC]                   # [BC, T, D] fp32
        xc = np.ascontiguousarray(
            xc.transpose(1, 0, 2).reshape(T, CB).astype(np.float16)
        )
        maps.append({"x": xc, "wall": wall})
    return maps


def assemble(res) -> np.ndarray:
    out = np.empty((B, T, D), dtype=np.float32)
    for i in range(NCORES):
        yc = res[i]["y"].astype(np.float32).reshape(T, BC, D)
        out[i * BC:(i + 1) * BC] = yc.transpose(1, 0, 2)
    return out


def kernel(x: np.ndarray) -> np.ndarray:
    res = run_bass_kernel_spmd(
        get_nc(), make_in_maps(x), list(range(NCORES))
    ).results
    return assemble(res)


# revision 13
# speedup vs baseline: 1.1804x; 1.1239x over previous
"""EMA (exponential moving average) linear recurrence on 8 trn2 NeuronCores.

y[0] = x[0]; y[t] = s*x[t] + (1-s)*y[t-1],  s = 0.3, x: (64, 4096, 256) fp32.

Algorithm: with a = 1-s = 0.7, a^128 ~ 1.6e-20, so history beyond 128 steps is
far below fp32 resolution. Chunk T into blocks of L=128 and write the scan as a
blocked FIR evaluated on the TensorEngine:

    y_c = M @ x_c + P @ x_{c-1}        (chunk 0: y_0 = M0 @ x_0)

with constant 128x128 matrices
    M[i,j]  = s * a^(i-j)   (j <= i),   M0 = M with column 0 scaled to a^i
    P[i,j]  = s * a^(i+128-j)           (dropped terms <= s*a^256 ~ 1e-40)

Sharding: batch B=64 split across the 8 cores (8 rows each); the recurrence is
along T only, so no cross-core communication is needed.

Precision/bandwidth: the harness gate is rel_err < 2e-2, so the kernel runs
fully in fp16: the host casts x to fp16 and transposes each core's shard to a
time-major [T, BC*D] layout (4 KiB contiguous DMA lines), the device computes
fp16 matmuls with fp32 PSUM accumulation, and writes y back as fp16; the host
upcasts/transposes to the fp32 output. End-to-end error vs the fp32 reference
is ~3e-4. This halves HBM traffic vs fp32 I/O (32 MiB/core total) and needs
just 2 PE passes/chunk, leaving the kernel DMA-bound near the ~332 GB/s/core
roofline.
"""
import numpy as np

import concourse.bacc as bacc
import concourse.mybir as mybir
from concourse import tile
from concourse.bass_utils import run_bass_kernel_spmd

S = 0.3
A = 1.0 - S
B, T, D = 64, 4096, 256
NCORES = 8
BC = B // NCORES          # 8 batch rows per core
L = 128                   # chunk length along T == matmul contraction dim
NCH = T // L              # 32 chunks
CB = BC * D               # 2048 free elements per chunk
NSL = CB // 512           # 4 matmul slices (one PSUM bank each)

f32 = mybir.dt.float32
f16 = mybir.dt.float16

_nc_cache = []


def _weights():
    i = np.arange(L, dtype=np.float64)[:, None]
    j = np.arange(L, dtype=np.float64)[None, :]
    M = np.where(j <= i, S * A ** (i - j), 0.0)
    M0 = M.copy()
    M0[:, 0] = A ** i[:, 0]
    P = S * A ** (i + L - j)
    # lhsT layout [K, M_out] = W.T, fp16
    return [np.ascontiguousarray(w.T.astype(np.float16)) for w in (M0, M, P)]


def _build():
    nc = bacc.Bacc("TRN2", target_bir_lowering=False, debug=False)
    x = nc.dram_tensor("x", [T, CB], f16, kind="ExternalInput").ap()
    # all three weight matrices in one tensor -> one DMA at kernel start
    wall = nc.dram_tensor("wall", [L, 3 * L], f16, kind="ExternalInput").ap()
    y = nc.dram_tensor("y", [T, CB], f16, kind="ExternalOutput").ap()

    with tile.TileContext(nc) as tc, \
         tc.tile_pool(name="w", bufs=1) as wpool, \
         tc.tile_pool(name="xs", bufs=20) as xpool, \
         tc.tile_pool(name="ys", bufs=6) as ypool, \
         tc.tile_pool(name="ps", bufs=2, space="PSUM") as pspool:
        wall_t = wpool.tile([L, 3 * L], f16)
        # first in the sync-ring queue: small, lands before chunk 0
        nc.sync.dma_start(wall_t[:], wall[:])
        wm0 = wall_t[:, 0 * L:1 * L]
        wm = wall_t[:, 1 * L:2 * L]
        wp = wall_t[:, 2 * L:3 * L]

        def load(c):
            xt = xpool.tile([L, CB], f16, name=f"xt{c}", tag="xt")
            src = x[c * L:(c + 1) * L, :]
            if c == 0:
                # chunk 0 gates PE start: pipeline it at 512-element slices
                for n in range(NSL):
                    sl = slice(n * 512, (n + 1) * 512)
                    nc.sync.dma_start(xt[:, sl], src[:, sl])
            else:
                nc.sync.dma_start(xt[:], src)
            return xt

        tiles = {0: load(0)}
        prev = None
        for c in range(NCH):
            # issue next chunk's load before this chunk's matmuls
            if c + 1 < NCH:
                tiles[c + 1] = load(c + 1)
            xt = tiles.pop(c)

            ps = pspool.tile([L, CB], f32)
            m = wm0 if c == 0 else wm
            # grouped by stationary weight to allow weight-load reuse
            for n in range(NSL):
                nc.tensor.matmul(
                    ps[:, n * 512:(n + 1) * 512], m,
                    xt[:, n * 512:(n + 1) * 512],
                    start=True, stop=(c == 0),
                )
            if c > 0:
                for n in range(NSL):
                    nc.tensor.matmul(
                        ps[:, n * 512:(n + 1) * 512], wp,
                        prev[:, n * 512:(n + 1) * 512],
                        start=False, stop=True,
                    )

            # evac PSUM->SBUF(fp16) on ACT/DVE alternating per chunk; the
            # output DMA trigger alternates between the scalar and gpsimd
            # DGE rings so neither ring's sequencer serializes both streams.
            yt = ypool.tile([L, CB], f16)
            dst = y[c * L:(c + 1) * L, :]
            if c >= NCH - 2:
                # drain the tail at slice granularity to cut end latency
                for n in range(NSL):
                    sl = slice(n * 512, (n + 1) * 512)
                    if n % 2 == 0:
                        nc.scalar.copy(yt[:, sl], ps[:, sl])
                        nc.scalar.dma_start(dst[:, sl], yt[:, sl])
                    else:
                        nc.vector.tensor_copy(yt[:, sl], ps[:, sl])
                        nc.gpsimd.dma_start(dst[:, sl], yt[:, sl])
            elif c % 2 == 0:
                nc.scalar.copy(yt[:], ps[:])
                nc.scalar.dma_start(dst, yt[:])
            else:
                nc.vector.tensor_copy(yt[:], ps[:])
                nc.gpsimd.dma_start(dst, yt[:])
            prev = xt
    nc.compile()
    return nc


def get_nc():
    if not _nc_cache:
        _nc_cache.append(_build())
    return _nc_cache[0]


def make_in_maps(x: np.ndarray):
    x = np.asarray(x)
    assert x.shape == (B, T, D)
    wall = np.ascontiguousarray(np.concatenate(_weights(), axis=1))
    maps = []
    for i in range(NCORES):
        xc = x[i * BC:(i + 1) * BC]                   # [BC, T, D] fp32
        xc = np.ascontiguousarray(
            xc.transpose(1, 0, 2).reshape(T, CB).astype(np.float16)
        )
        maps.append({"x": xc, "wall": wall})
    return maps


def assemble(res) -> np.ndarray:
    out = np.empty((B, T, D), dtype=np.float32)
    for i in range(NCORES):
        yc = res[i]["y"].astype(np.float32).reshape(T, BC, D)
        out[i * BC:(i + 1) * BC] = yc.transpose(1, 0, 2)
    return out


def kernel(x: np.ndarray) -> np.ndarray:
    res = run_bass_kernel_spmd(
        get_nc(), make_in_maps(x), list(range(NCORES))
    ).results
    return assemble(res)
